# revision 1
# baseline (speedup 1.0000x reference)
"""DenseCRF loss kernel for Trainium2, data-parallel over batch on 8 NeuronCores.

reference:
  seg = bilinear_resize(segmentations, 128->64)            # [N,K,64,64]
  f_i = [x_i/50, y_i/50, r_i/15, g_i/15, b_i/15]           # 5-dim bilateral feature
  W_ij = exp(-0.5*|f_i - f_j|^2)                           # [P,P], P=4096
  loss = WEIGHT * (-sum_k s_k^T W s_k) / N

Per core (1 image): W block = exp(G - q_i - q_j) with G the 5-d Gram matrix.
G is computed on the TensorEngine as a 22-row bf16 matmul where every feature is
split hi/lo into two bf16 values (bf16 products are exact in the fp32 PSUM
accumulator, so the only error is the tiny split residual). -q_i rides two bf16
aux rows; -q_j is the fp32 per-partition bias of the Exp activation. The exp'd
block (bf16) is contracted against the resized segmentation with PSUM
accumulation; a DVE multiply+reduce forms the scalar, host sums 8 cores.

Row pairing of the 22-row contraction (FA row r pairs with FB row r):
  FA: [H5 | H5 | L5 | L5 | 1 1]     H5 = [pxh pyh fh_r fh_g fh_b]
  FB: [H5 | L5 | H5 | L5 | -qh -ql] L5 = [pxl pyl fl_r fl_g fl_b]
"""

import sys

sys.path.insert(0, "/opt/trn_rl_repo")

import numpy as np
import ml_dtypes

import concourse.bass as bass
import concourse.tile as tile
from concourse import bacc, bass_isa, mybir
from concourse.bass_utils import run_bass_kernel_spmd

F32 = mybir.dt.float32
BF16 = mybir.dt.bfloat16
AF = mybir.ActivationFunctionType
ALU = mybir.AluOpType
BF = ml_dtypes.bfloat16

N, C, K = 8, 3, 21
H, W = 64, 64
P = H * W  # 4096
SIGMA_RGB = 15.0
SXY = 100.0 * 0.5  # sigma_xy * scale
WEIGHT = 1e-8
NB = 32  # 128-row chunks of P
NG = 8  # 512-col groups of P


def _resize_matrix():
    """[64,128] weights of jax.image.resize(..., method='bilinear') along one dim
    (triangle kernel, antialias=True, scale=0.5, renormalized)."""
    y = np.arange(128, dtype=np.float64)[:, None]
    sample = 2.0 * np.arange(64, dtype=np.float64)[None, :] + 0.5
    w = np.maximum(0.0, 1.0 - 0.5 * np.abs(y - sample))
    w = w / w.sum(axis=0, keepdims=True)
    return np.ascontiguousarray(w.T.astype(np.float32))  # [64,128]


def _consts():
    R = _resize_matrix()  # [64,128]
    rtf = np.ascontiguousarray(R.T)  # [128,64] f32
    rtb = rtf.astype(BF)
    idf = np.eye(128, dtype=np.float32)
    idb = idf.astype(BF)
    i = np.arange(P, dtype=np.float32)
    px = (i % 64).astype(np.float32) / np.float32(SXY)
    py = (i // 64).astype(np.float32) / np.float32(SXY)
    pos = np.stack([px, py])  # [2,P] f32
    ph2 = pos.astype(BF)
    pl2 = (pos - ph2.astype(np.float32)).astype(BF)
    pf2 = ph2.astype(np.float32) + pl2.astype(np.float32)  # exact f~ for positions
    # constant skeletons of FA/FB: position + ones rows, zeros where the
    # color / q rows get DMA'd in on-device
    fabA = np.zeros((22, P), dtype=BF)
    fabB = np.zeros((22, P), dtype=BF)
    fabA[0:2] = ph2
    fabA[5:7] = ph2
    fabA[10:12] = pl2
    fabA[15:17] = pl2
    fabA[20:22] = np.ones((2, P), dtype=BF)
    fabB[0:2] = ph2
    fabB[10:12] = ph2
    fabB[5:7] = pl2
    fabB[15:17] = pl2
    neghalf5 = np.full((5, 1), -0.5, dtype=np.float32)
    return dict(rtf=rtf, rtb=rtb, idf=idf, idb=idb, fabA=fabA, fabB=fabB,
                pf2=pf2, neghalf5=neghalf5)


def _build():
    nc = bacc.Bacc()
    images_d = nc.dram_tensor("images", [C, H, W], F32, kind="ExternalInput")
    seg_d = nc.dram_tensor("segmentations", [K, 128, 128], F32, kind="ExternalInput")
    rtf_d = nc.dram_tensor("rtf", [128, 64], F32, kind="ExternalInput")
    rtb_d = nc.dram_tensor("rtb", [128, 64], BF16, kind="ExternalInput")
    idf_d = nc.dram_tensor("idf", [128, 128], F32, kind="ExternalInput")
    idb_d = nc.dram_tensor("idb", [128, 128], BF16, kind="ExternalInput")
    fabA_d = nc.dram_tensor("fabA", [22, P], BF16, kind="ExternalInput")
    fabB_d = nc.dram_tensor("fabB", [22, P], BF16, kind="ExternalInput")
    pf2_d = nc.dram_tensor("pf2", [2, P], F32, kind="ExternalInput")
    nh5_d = nc.dram_tensor("neghalf5", [5, 1], F32, kind="ExternalInput")
    out_d = nc.dram_tensor("out", [1], F32, kind="ExternalOutput")

    with tile.TileContext(nc) as tc:
        with tc.tile_pool(name="persist", bufs=1) as pp:
            FA = pp.tile([22, P], BF16, tag="FA")
            FB = pp.tile([22, P], BF16, tag="FB")
            qcol = pp.tile([128, NB], F32, tag="qcol")
            Ftil = pp.tile([5, P], F32, tag="Ftil")
            Fsq = pp.tile([5, P], F32, tag="Fsq")
            q2ar = pp.tile([5, P], F32, tag="q2ar")
            fh3 = pp.tile([3, P], BF16, tag="fh3")
            fl3 = pp.tile([3, P], BF16, tag="fl3")
            qh1 = pp.tile([1, P], BF16, tag="qh1")
            ql1 = pp.tile([1, P], BF16, tag="ql1")
            nh5_s = pp.tile([5, 1], F32, tag="nh5")
            Srow = pp.tile([K, P], F32, tag="Srow")
            STt = pp.tile([128, NB * K], BF16, tag="STt")
            rtf_s = pp.tile([128, 64], F32, tag="rtf")
            rtb_s = pp.tile([128, 64], BF16, tag="rtb")
            idf_s = pp.tile([128, 128], F32, tag="idf")
            idb_s = pp.tile([128, 128], BF16, tag="idb")
            img_s = pp.tile([C, P], F32, tag="img")
            seg_s = pp.tile([128, K * 128], F32, tag="seg")
            A_sb = pp.tile([64, K * 128], BF16, tag="A_sb")
            At = pp.tile([128, K * 64], BF16, tag="At")
            partials = pp.tile([K, NG], F32, tag="partials")
            pr1 = pp.tile([K, 1], F32, tag="pr1")
            tot = pp.tile([K, 1], F32, tag="tot")
            osb = pp.tile([1, 1], F32, tag="osb")

            # ---- load inputs / constants ----
            # DMA issue cost (~1.7us each) serializes per queue: spread over the
            # three DMA-capable queues. The q-chain (images -> colors -> Fsq ->
            # all-reduce -> qh/ql -> FB rows) is the critical path, so the Pool
            # queue carries only seg (the all-reduce must run early) and the
            # ACT queue runs its compute before its replica DMAs.
            dma = nc.sync.dma_start
            dmag = nc.gpsimd.dma_start
            dmaa = nc.scalar.dma_start
            inv15 = float(np.float32(1.0) / np.float32(SIGMA_RGB))
            # Queue layout: images first on the gpsimd queue (it gates the
            # q-chain), then resize inputs in usage order; FA/q-row writes ride
            # the sync queue tail; FB color replicas ride the ACT queue after
            # its compute.
            dmag(img_s[:], images_d.rearrange("c h w -> c (h w)"))
            dmag(seg_s[:], seg_d.rearrange("k y x -> y k x"))
            dmag(FB[:], fabB_d[:])
            dmag(rtf_s[:], rtf_d[:])
            dmag(idb_s[:], idb_d[:])
            dmag(rtb_s[:], rtb_d[:])
            dmag(idf_s[:], idf_d[:])
            dma(Ftil[3:5, :], pf2_d[:])
            dma(FA[:], fabA_d[:])
            dma(nh5_s[:], nh5_d[:])

            # color features (hi/lo split of img/15) at partition 0; engines
            # cannot address partition offsets that aren't multiples of 32, so
            # rows are staged and DMA'd into the FA/FB row slots.
            inv15 = float(np.float32(1.0) / np.float32(SIGMA_RGB))
            nc.scalar.activation(fh3[:], img_s[:], AF.Copy, scale=inv15)  # fh
            nc.scalar.activation(Ftil[0:3, :], img_s[:], AF.Copy, scale=inv15)
            nc.vector.scalar_tensor_tensor(
                fl3[:], img_s[:], inv15, fh3[:], ALU.mult, ALU.subtract
            )  # fl = img/15 - fh

            # q = 0.5|f~|^2  (Ftil rows: [colors | positions]; sum is order-free)
            nc.vector.tensor_mul(Fsq[:], Ftil[:], Ftil[:])
            nc.gpsimd.partition_all_reduce(q2ar[:], Fsq[:], 5, bass_isa.ReduceOp.add)
            q2row = q2ar[0:1, :]
            nc.scalar.activation(qh1[:], q2row, AF.Copy, scale=-0.5)  # -qh
            nc.vector.scalar_tensor_tensor(
                ql1[:], q2row, -0.5, qh1[:], ALU.mult, ALU.subtract
            )  # -ql = -q - (-qh)
            dma(FB[20:21, :], qh1[:])
            dma(FB[21:22, :], ql1[:])
            dmaa(FB[2:5, :], fh3[:])
            dmaa(FB[12:15, :], fh3[:])
            dmaa(FB[7:10, :], fl3[:])
            dmaa(FB[17:20, :], fl3[:])
            dma(FA[2:5, :], fh3[:])
            dma(FA[7:10, :], fh3[:])
            dma(FA[12:15, :], fl3[:])
            dma(FA[17:20, :], fl3[:])

            with tc.tile_pool(name="prep_ps", bufs=8, space="PSUM") as pps:
                # ---- resize: rows (contract Y) ----
                # (emitted before the q-dependent matmuls: PE executes in order,
                # and resize inputs arrive long before Fsq is ready)
                for c0 in range(0, K * 128, 512):
                    c1 = min(c0 + 512, K * 128)
                    aps = pps.tile([64, 512], F32, tag="ps", name=f"aps{c0}")
                    nc.tensor.matmul(
                        aps[:, : c1 - c0], rtf_s[:], seg_s[:, c0:c1],
                        start=True, stop=True,
                    )
                    nc.vector.tensor_copy(A_sb[:, c0:c1], aps[:, : c1 - c0])
                # transpose per class: At[X, (k,y')]
                for k0 in range(0, K, 8):
                    k1 = min(k0 + 8, K)
                    tps = pps.tile([128, 64 * 8], BF16, tag="ps", name=f"tps{k0}")
                    for k in range(k0, k1):
                        nc.tensor.transpose(
                            tps[:, (k - k0) * 64 : (k - k0 + 1) * 64],
                            A_sb[0:64, k * 128 : (k + 1) * 128], idb_s[0:64, 0:64]
                        )
                    nc.vector.tensor_copy(
                        At[:, k0 * 64 : k1 * 64], tps[:, : (k1 - k0) * 64]
                    )
                # cols (contract X): Srow[k, y'*64+x']
                at3 = At[:, :].rearrange("x (k y) -> x k y", k=K, y=64)
                for yb in range(8):
                    sps = pps.tile([K, 512], F32, tag="ps", name=f"sps{yb}")
                    for yl in range(8):
                        yp = yb * 8 + yl
                        nc.tensor.matmul(
                            sps[:, yl * 64 : (yl + 1) * 64],
                            at3[:, :, yp], rtb_s[:],
                            start=True, stop=True,
                        )
                    nc.vector.tensor_copy(Srow[:, yb * 512 : (yb + 1) * 512], sps[:])
                # STt chunks: [128, 21] per b (bf16, acc-matmul weights)
                for b0 in range(0, NB, 8):
                    t2 = pps.tile([128, K * 8], F32, tag="ps", name=f"t2_{b0}")
                    for b in range(b0, b0 + 8):
                        nc.tensor.transpose(
                            t2[:, (b - b0) * K : (b - b0 + 1) * K],
                            Srow[:, b * 128 : (b + 1) * 128], idf_s[0:K, 0:K]
                        )
                    nc.vector.tensor_copy(STt[:, b0 * K : (b0 + 8) * K], t2[:])

                # qcol[:, b] = -q for chunk b (fp32, used as Exp bias)
                qps = pps.tile([128, NB], F32, tag="ps", name="qps")
                for b in range(NB):
                    nc.tensor.matmul(
                        qps[:, b : b + 1],
                        Fsq[:, b * 128 : (b + 1) * 128],
                        nh5_s[:],
                        start=True, stop=True,
                    )
                nc.vector.tensor_copy(qcol[:], qps[:])


            # ---- main loop: 4 passes x 32 chunks x one 1024-wide exp unit ----
            with (
                tc.tile_pool(name="gps", bufs=3, space="PSUM") as gps,
                tc.tile_pool(name="accps", bufs=2, space="PSUM") as accps,
                tc.tile_pool(name="ep", bufs=8) as ep,
                tc.tile_pool(name="finp", bufs=2) as finp,
            ):
                for p in range(4):
                    accs = [
                        accps.tile([K, 512], F32, tag="acc", name=f"acc{p}_{gi}")
                        for gi in range(2)
                    ]
                    pend = []  # software pipeline: acc-matmuls lag one chunk
                    for b in range(NB):
                        fa_b = FA[:, b * 128 : (b + 1) * 128]
                        g0 = p * 2
                        if len(pend) > 1:
                            pb, pet = pend.pop(0)
                            for gi in range(2):
                                nc.tensor.matmul(
                                    accs[gi][:],
                                    STt[:, pb * K : (pb + 1) * K],
                                    pet[:, gi * 512 : (gi + 1) * 512],
                                    start=(pb == 0), stop=(pb == NB - 1),
                                )
                        gt = gps.tile([128, 1024], F32, tag="g", name=f"g{p}_{b}")
                        nc.tensor.matmul(
                            gt[:, 0:512], fa_b,
                            FB[:, g0 * 512 : (g0 + 1) * 512],
                            start=True, stop=True,
                        )
                        nc.tensor.matmul(
                            gt[:, 512:1024], fa_b,
                            FB[:, (g0 + 1) * 512 : (g0 + 2) * 512],
                            start=True, stop=True,
                        )
                        et = ep.tile([128, 1024], BF16, tag="e", name=f"e{p}_{b}")
                        nc.scalar.activation(
                            et[:], gt[:], AF.Exp, bias=qcol[:, b : b + 1]
                        )
                        pend.append((b, et))
                    for pb, pet in pend:
                        for gi in range(2):
                            nc.tensor.matmul(
                                accs[gi][:],
                                STt[:, pb * K : (pb + 1) * K],
                                pet[:, gi * 512 : (gi + 1) * 512],
                                start=(pb == 0), stop=(pb == NB - 1),
                            )
                    # loss partials: sum_k,i acc[k,i] * Srow[k,i]
                    for gi in range(2):
                        g = p * 2 + gi
                        sc = finp.tile([K, 512], F32, tag="sc", name=f"sc{p}_{gi}")
                        nc.vector.tensor_mul(
                            sc[:], accs[gi][:], Srow[:, g * 512 : (g + 1) * 512]
                        )
                        nc.vector.tensor_reduce(
                            partials[:, g : g + 1], sc[:], mybir.AxisListType.X, ALU.add
                        )

                nc.vector.tensor_reduce(pr1[:], partials[:], mybir.AxisListType.X, ALU.add)
                nc.gpsimd.partition_all_reduce(tot[:], pr1[:], K, bass_isa.ReduceOp.add)
                nc.scalar.activation(osb[:], tot[0:1, :], AF.Copy, scale=float(-WEIGHT / N))
                nc.sync.dma_start(out_d[:], osb[:])

    nc.finalize()
    return nc


_CACHE = {}


def _get_nc():
    if "nc" not in _CACHE:
        _CACHE["nc"] = _build()
    return _CACHE["nc"]


def kernel(images: np.ndarray, segmentations: np.ndarray) -> np.ndarray:
    images = np.ascontiguousarray(np.asarray(images, dtype=np.float32))
    segmentations = np.ascontiguousarray(np.asarray(segmentations, dtype=np.float32))
    assert images.shape == (N, C, H, W) and segmentations.shape == (N, K, 128, 128)
    nc = _get_nc()
    consts = _consts()
    in_maps = [
        {"images": images[n], "segmentations": segmentations[n], **consts}
        for n in range(N)
    ]
    res = run_bass_kernel_spmd(nc, in_maps, list(range(N)))
    total = sum(float(res.results[n]["out"][0]) for n in range(N))
    return np.array([total], dtype=np.float32)


if __name__ == "__main__":
    rng = np.random.RandomState(0)
    img = rng.rand(N, C, H, W).astype(np.float32) * 255.0
    seg = rng.rand(N, K, 128, 128).astype(np.float32)
    print(kernel(img, seg))



# revision 3
# speedup vs baseline: 1.7745x; 1.7745x over previous
"""DenseCRF loss kernel for Trainium2, data-parallel over batch on 8 NeuronCores.

reference:
  seg = bilinear_resize(segmentations, 128->64)            # [N,K,64,64]
  f_i = [x_i/50, y_i/50, r_i/15, g_i/15, b_i/15]           # 5-dim bilateral feature
  W_ij = exp(-0.5*|f_i - f_j|^2)                           # [P,P], P=4096
  loss = WEIGHT * (-sum_k s_k^T W s_k) / N

Per core (1 image): W block = exp(G - q_i - q_j) with G the 5-d Gram matrix.
G is computed on the TensorEngine as a 22-row bf16 matmul where every feature is
split hi/lo into two bf16 values (bf16 products are exact in the fp32 PSUM
accumulator, so the only error is the tiny split residual). -q_j rides two bf16
aux rows of FB; -q_i is the fp32 per-partition bias of the Exp activation. The
exp'd block (bf16) is contracted against the resized segmentation with PSUM
accumulation; a fused DVE multiply+reduce forms the scalar, host sums 8 cores.

W is symmetric: only diagonal + strictly-upper 512x512 units are computed.
Strictly-upper blocks get a factor 2, folded either into a second exp bias
qcol2 = qcol + ln2 (both halves of a 1024-wide block upper) or into a doubled
seg-transpose STt2 (only the right half upper).

The whole q/feature prep runs in position-partition layout [128, chunk*feat]
(ops cost ~100ns instead of ~4us in row layout); FA/FB are assembled by
per-chunk PE transposes of feature-major tiles, and qcol IS the nq tile.

Row pairing of the 22-row contraction (FA row r pairs with FB row r):
  FA: [H5 | H5 | L5 | L5 | 1 1]     H5 = [pxh pyh fh_r fh_g fh_b]
  FB: [H5 | L5 | H5 | L5 | -qh -ql] L5 = [pxl pyl fl_r fl_g fl_b]
"""

import sys

sys.path.insert(0, "/opt/trn_rl_repo")

import numpy as np
import ml_dtypes

import concourse.bass as bass
import concourse.tile as tile
from concourse import bacc, bass_isa, mybir
from concourse.bass_utils import run_bass_kernel_spmd

F32 = mybir.dt.float32
F32R = mybir.dt.float32r
BF16 = mybir.dt.bfloat16
AF = mybir.ActivationFunctionType
ALU = mybir.AluOpType
BF = ml_dtypes.bfloat16

N, C, K = 8, 3, 21
H, W = 64, 64
P = H * W  # 4096
SIGMA_RGB = 15.0
SXY = 100.0 * 0.5  # sigma_xy * scale
WEIGHT = 1e-8
NB = 32  # 128-row chunks of P
NF = 22  # feature rows of the Gram contraction
LN2 = float(np.log(2.0))


def _resize_matrix():
    """[64,128] weights of jax.image.resize(..., method='bilinear') along one dim
    (triangle kernel, antialias=True, scale=0.5, renormalized)."""
    y = np.arange(128, dtype=np.float64)[:, None]
    sample = 2.0 * np.arange(64, dtype=np.float64)[None, :] + 0.5
    w = np.maximum(0.0, 1.0 - 0.5 * np.abs(y - sample))
    w = w / w.sum(axis=0, keepdims=True)
    return np.ascontiguousarray(w.T.astype(np.float32))  # [64,128]


def _consts():
    R = _resize_matrix()  # [64,128]
    rtf = np.ascontiguousarray(R.T)  # [128,64] f32
    rtb = rtf.astype(BF)
    idf = np.eye(128, dtype=np.float32)
    idb = idf.astype(BF)
    i = np.arange(P, dtype=np.float32)
    px = (i % 64).astype(np.float32) / np.float32(SXY)
    py = (i // 64).astype(np.float32) / np.float32(SXY)
    pos = np.stack([px, py])  # [2,P] f32
    ph2 = pos.astype(BF)
    pl2 = (pos - ph2.astype(np.float32)).astype(BF)
    pf2 = ph2.astype(np.float32) + pl2.astype(np.float32)  # exact f~ for positions
    # feature-major skeletons [r=128, (b, f)] with the constant slots filled:
    # position hi/lo replicas (+ ones rows for FA); color/q slots zero.
    phr = ph2.reshape(2, NB, 128)  # [2, b, r]
    plr = pl2.reshape(2, NB, 128)
    fcbA = np.zeros((128, NB, NF), dtype=BF)
    fcbB = np.zeros((128, NB, NF), dtype=BF)
    for d in range(2):
        hi = phr[d].T  # [r, b]
        lo = plr[d].T
        fcbA[:, :, 0 + d] = hi
        fcbA[:, :, 5 + d] = hi
        fcbA[:, :, 10 + d] = lo
        fcbA[:, :, 15 + d] = lo
        fcbB[:, :, 0 + d] = hi
        fcbB[:, :, 5 + d] = lo
        fcbB[:, :, 10 + d] = hi
        fcbB[:, :, 15 + d] = lo
    fcbA[:, :, 20:22] = np.ones((128, NB, 2), dtype=BF)
    fcbA = np.ascontiguousarray(fcbA.reshape(128, NB * NF))
    fcbB = np.ascontiguousarray(fcbB.reshape(128, NB * NF))
    # -0.5*(px^2+py^2) in [r, b] layout
    nqpos = (-0.5 * (pf2[0] ** 2 + pf2[1] ** 2)).reshape(NB, 128).T
    nqpos = np.ascontiguousarray(nqpos.astype(np.float32))
    return dict(rtf=rtf, rtb=rtb, idf=idf, idb=idb, fcbA=fcbA, fcbB=fcbB,
                nqpos=nqpos)


def _build():
    nc = bacc.Bacc()
    images_d = nc.dram_tensor("images", [C, H, W], F32, kind="ExternalInput")
    seg_d = nc.dram_tensor("segmentations", [K, 128, 128], F32, kind="ExternalInput")
    rtf_d = nc.dram_tensor("rtf", [128, 64], F32, kind="ExternalInput")
    rtb_d = nc.dram_tensor("rtb", [128, 64], BF16, kind="ExternalInput")
    idf_d = nc.dram_tensor("idf", [128, 128], F32, kind="ExternalInput")
    idb_d = nc.dram_tensor("idb", [128, 128], BF16, kind="ExternalInput")
    fcbA_d = nc.dram_tensor("fcbA", [128, NB * NF], BF16, kind="ExternalInput")
    fcbB_d = nc.dram_tensor("fcbB", [128, NB * NF], BF16, kind="ExternalInput")
    nqpos_d = nc.dram_tensor("nqpos", [128, NB], F32, kind="ExternalInput")
    out_d = nc.dram_tensor("out", [1], F32, kind="ExternalOutput")

    inv15 = float(np.float32(1.0) / np.float32(SIGMA_RGB))

    with tile.TileContext(nc) as tc:
        with tc.tile_pool(name="persist", bufs=1) as pp:
            FA = pp.tile([NF, P], BF16, tag="FA")
            FB = pp.tile([NF, P], BF16, tag="FB")
            # q/feature chain, [r, (c b)] / [r, b] layouts
            img_cb = pp.tile([3 * NB, 128], F32, tag="img_cb")
            img_rc = pp.tile([128, 3 * NB], F32, tag="img_rc")
            ftil = pp.tile([128, 3 * NB], F32, tag="ftil")
            fh = pp.tile([128, 3 * NB], BF16, tag="fh")
            fl = pp.tile([128, 3 * NB], BF16, tag="fl")
            fsq = pp.tile([128, 3 * NB], F32, tag="fsq")
            csum = pp.tile([128, NB], F32, tag="csum")
            qcol = pp.tile([128, NB], F32, tag="qcol")  # = -q (exp bias)
            qcol2 = pp.tile([128, NB], F32, tag="qcol2")
            nqh = pp.tile([128, NB], BF16, tag="nqh")
            nql = pp.tile([128, NB], BF16, tag="nql")
            nqpos_s = pp.tile([128, NB], F32, tag="nqpos")
            fcbA_s = pp.tile([128, NB * NF], BF16, tag="fcbA")
            fcbB_s = pp.tile([128, NB * NF], BF16, tag="fcbB")
            # resize pipeline
            rtf_s = pp.tile([128, 64], F32, tag="rtf")
            rtb_s = pp.tile([128, 64], BF16, tag="rtb")
            idf_s = pp.tile([128, 128], F32, tag="idf")
            idb_s = pp.tile([128, 128], BF16, tag="idb")
            seg_s = pp.tile([128, K * 128], F32, tag="seg")
            A_sb = pp.tile([64, K * 128], BF16, tag="A_sb")
            At = pp.tile([128, K * 64], BF16, tag="At")
            Srow = pp.tile([K, P], F32, tag="Srow")
            STt = pp.tile([128, NB * K], BF16, tag="STt")
            STt2 = pp.tile([128, NB * K], BF16, tag="STt2")
            # loss tail
            partials = pp.tile([K, 8], F32, tag="partials")
            pr1 = pp.tile([K, 1], F32, tag="pr1")
            tot = pp.tile([K, 1], F32, tag="tot")
            osb = pp.tile([1, 1], F32, tag="osb")

            # ---- DMAs ----
            # Pool/SWDGE: img only (heads the q-chain; transfer is tiny).
            nc.gpsimd.dma_start(img_cb[:], images_d.rearrange(
                "c (b h2) w -> (c b) (h2 w)", h2=2))
            # ACT/HWDGE: idf (small, needed by the img transpose), then the
            # big seg transfer -- its DMA_ENGINES grab must come after img's.
            nc.scalar.dma_start(idf_s[:], idf_d[:])
            nc.scalar.dma_start(seg_s[:], seg_d.rearrange("k y x -> y k x"))
            # SP/HWDGE: everything else, usage order.
            nc.sync.dma_start(fcbB_s[:], fcbB_d[:])
            nc.sync.dma_start(fcbA_s[:], fcbA_d[:])
            nc.sync.dma_start(nqpos_s[:], nqpos_d[:])
            nc.sync.dma_start(idb_s[:], idb_d[:])
            nc.sync.dma_start(rtf_s[:], rtf_d[:])
            nc.sync.dma_start(rtb_s[:], rtb_d[:])

            with (
                tc.tile_pool(name="gps", bufs=2, space="PSUM") as gps,
                tc.tile_pool(name="accps", bufs=2, space="PSUM") as accps,
                tc.tile_pool(name="pps", bufs=2, space="PSUM") as pps,
                tc.tile_pool(name="ep", bufs=8) as ep,
                tc.tile_pool(name="scp", bufs=2) as scp,
            ):
                # ---- q chain ----
                ips = pps.tile([128, 3 * NB], F32, tag="ps", name="ips")
                nc.tensor.transpose(ips[:], img_cb[:], idf_s[0:3 * NB, 0:3 * NB])
                nc.gpsimd.tensor_copy(img_rc[:], ips[:])
                nc.scalar.activation(fh[:], img_rc[:], AF.Copy, scale=inv15)
                nc.vector.tensor_scalar_mul(ftil[:], img_rc[:], inv15)
                nc.vector.scalar_tensor_tensor(
                    fl[:], img_rc[:], inv15, fh[:], ALU.mult, ALU.subtract)
                nc.vector.tensor_mul(fsq[:], ftil[:], ftil[:])
                nc.vector.tensor_add(csum[:], fsq[:, 0:NB], fsq[:, NB:2 * NB])
                nc.vector.tensor_add(csum[:], csum[:], fsq[:, 2 * NB:3 * NB])
                # qcol = -q = -0.5*colorsq + nqpos
                nc.vector.scalar_tensor_tensor(
                    qcol[:], csum[:], -0.5, nqpos_s[:], ALU.mult, ALU.add)
                nc.scalar.activation(nqh[:], qcol[:], AF.Copy)
                nc.vector.scalar_tensor_tensor(
                    nql[:], qcol[:], 1.0, nqh[:], ALU.mult, ALU.subtract)
                nc.gpsimd.tensor_scalar_add(qcol2[:], qcol[:], LN2)

                # ---- assemble feature-major tiles (color + q slots) ----
                a3 = fcbA_s[:].rearrange("r (b f) -> r b f", f=NF)
                b3 = fcbB_s[:].rearrange("r (b f) -> r b f", f=NF)
                fh3 = fh[:].rearrange("r (c b) -> r b c", c=3)
                fl3 = fl[:].rearrange("r (c b) -> r b c", c=3)
                nc.gpsimd.tensor_copy(a3[:, :, 2:5], fh3)
                nc.gpsimd.tensor_copy(a3[:, :, 7:10], fh3)
                nc.gpsimd.tensor_copy(a3[:, :, 12:15], fl3)
                nc.gpsimd.tensor_copy(a3[:, :, 17:20], fl3)
                nc.gpsimd.tensor_copy(b3[:, :, 2:5], fh3)
                nc.gpsimd.tensor_copy(b3[:, :, 12:15], fh3)
                nc.gpsimd.tensor_copy(b3[:, :, 7:10], fl3)
                nc.gpsimd.tensor_copy(b3[:, :, 17:20], fl3)
                nc.gpsimd.tensor_copy(b3[:, :, 20:21], nqh[:].unsqueeze(2))
                nc.gpsimd.tensor_copy(b3[:, :, 21:22], nql[:].unsqueeze(2))

                # ---- FA/FB columns via per-chunk PE transposes ----
                def emit_fab(b0, b1):
                    for src, dst, nm in ((a3, FA, "A"), (b3, FB, "B")):
                        fps = pps.tile([NF, 8 * 128], BF16, tag="ps",
                                       name=f"f{nm}{b0}")
                        for b in range(b0, b1):
                            nc.tensor.transpose(
                                fps[:, (b - b0) * 128:(b - b0 + 1) * 128],
                                src[:, b, :], idb_s[:, 0:128])
                        nc.gpsimd.tensor_copy(
                            dst[:, b0 * 128:b1 * 128], fps[:, :(b1 - b0) * 128])

                emit_fab(0, 8)

                # ---- main loop plumbing ----
                def pass_chunks(p):
                    ch = []
                    for b in range(8 * p):
                        ch.append((b, 2, qcol2, [(0, STt), (1, STt)]))
                    for b in range(8 * p, 8 * p + 4):
                        ch.append((b, 2, qcol, [(0, STt), (1, STt2)]))
                    for b in range(8 * p + 4, 8 * p + 8):
                        ch.append((b, 1, qcol, [(1, STt)]))
                    return ch

                def emit_gram_exp(p, b, width, biast):
                    g0 = 2 * p
                    fa_b = FA[:, b * 128:(b + 1) * 128]
                    gt = gps.tile([128, 1024], F32, tag="g", name=f"g{p}_{b}")
                    if width == 2:
                        nc.tensor.matmul(
                            gt[:, 0:512], fa_b, FB[:, g0 * 512:(g0 + 1) * 512],
                            start=True, stop=True)
                        nc.tensor.matmul(
                            gt[:, 512:1024], fa_b,
                            FB[:, (g0 + 1) * 512:(g0 + 2) * 512],
                            start=True, stop=True)
                    else:
                        nc.tensor.matmul(
                            gt[:, 0:512], fa_b,
                            FB[:, (g0 + 1) * 512:(g0 + 2) * 512],
                            start=True, stop=True)
                    et = ep.tile([128, 1024], BF16, tag="e", name=f"e{p}_{b}")
                    nc.scalar.activation(
                        et[:, 0:width * 512], gt[:, 0:width * 512], AF.Exp,
                        bias=biast[:, b:b + 1])
                    return et

                def emit_accs(p, accs, b, width, targets, et):
                    last = (8 * p + 3, 8 * p + 7)
                    for gl, stt in targets:
                        sl = et[:, gl * 512:(gl + 1) * 512] if width == 2 \
                            else et[:, 0:512]
                        nc.tensor.matmul(
                            accs[gl][:], stt[:, b * K:(b + 1) * K], sl,
                            start=(b == 0), stop=(b == last[gl]))

                def emit_finals(p, accs):
                    for gl in range(2):
                        g = 2 * p + gl
                        sc = scp.tile([K, 512], F32, tag="sc", name=f"sc{p}_{gl}")
                        nc.vector.tensor_tensor_reduce(
                            sc[:], accs[gl][:], Srow[:, g * 512:(g + 1) * 512],
                            1.0, 0.0, ALU.mult, ALU.add,
                            partials[:, g:g + 1])

                # pass 0: gram+exp for all 8 chunks first; the remaining FA/FB
                # batches and the resize pipeline run on PE while ACT exps.
                accs0 = [accps.tile([K, 512], F32, tag="acc", name=f"acc0_{i}")
                         for i in range(2)]
                ch0 = pass_chunks(0)
                ets0 = [emit_gram_exp(0, b, w, bia) for b, w, bia, _ in ch0]

                for b0 in range(8, NB, 8):
                    emit_fab(b0, b0 + 8)

                # ---- resize pipeline (PE matmuls + Pool copies) ----
                for c0 in range(0, K * 128, 512):
                    c1 = min(c0 + 512, K * 128)
                    aps = pps.tile([64, 512], F32, tag="ps", name=f"aps{c0}")
                    nc.tensor.matmul(
                        aps[:, :c1 - c0], rtf_s[:].bitcast(F32R),
                        seg_s[:, c0:c1].bitcast(F32R), start=True, stop=True)
                    nc.gpsimd.tensor_copy(A_sb[:, c0:c1], aps[:, :c1 - c0])
                for k0 in range(0, K, 8):
                    k1 = min(k0 + 8, K)
                    tps = pps.tile([128, 64 * 8], BF16, tag="ps", name=f"tps{k0}")
                    for k in range(k0, k1):
                        nc.tensor.transpose(
                            tps[:, (k - k0) * 64:(k - k0 + 1) * 64],
                            A_sb[0:64, k * 128:(k + 1) * 128], idb_s[0:64, 0:64])
                    nc.gpsimd.tensor_copy(
                        At[:, k0 * 64:k1 * 64], tps[:, :(k1 - k0) * 64])
                at3 = At[:, :].rearrange("x (k y) -> x k y", k=K, y=64)
                for yb in range(8):
                    sps = pps.tile([K, 512], F32, tag="ps", name=f"sps{yb}")
                    for yl in range(8):
                        yp = yb * 8 + yl
                        nc.tensor.matmul(
                            sps[:, yl * 64:(yl + 1) * 64],
                            at3[:, :, yp], rtb_s[:], start=True, stop=True)
                    nc.gpsimd.tensor_copy(Srow[:, yb * 512:(yb + 1) * 512], sps[:])
                for b0 in range(0, NB, 8):
                    t2 = pps.tile([128, K * 8], F32, tag="ps", name=f"t2_{b0}")
                    for b in range(b0, b0 + 8):
                        nc.tensor.transpose(
                            t2[:, (b - b0) * K:(b - b0 + 1) * K],
                            Srow[:, b * 128:(b + 1) * 128], idf_s[0:K, 0:K])
                    nc.gpsimd.tensor_copy(STt[:, b0 * K:(b0 + 8) * K], t2[:])
                nc.gpsimd.tensor_scalar_mul(STt2[:], STt[:], 2.0)

                # pass-0 accs (need STt), then finals
                for (b, w, bia, tg), et in zip(ch0, ets0):
                    emit_accs(0, accs0, b, w, tg, et)
                emit_finals(0, accs0)

                # passes 1..3: lag-1 software pipeline
                for p in range(1, 4):
                    accs = [accps.tile([K, 512], F32, tag="acc",
                                       name=f"acc{p}_{i}") for i in range(2)]
                    pend = []
                    for b, w, bia, tg in pass_chunks(p):
                        if len(pend) > 1:
                            pb, pw, ptg, pet = pend.pop(0)
                            emit_accs(p, accs, pb, pw, ptg, pet)
                        et = emit_gram_exp(p, b, w, bia)
                        pend.append((b, w, tg, et))
                    for pb, pw, ptg, pet in pend:
                        emit_accs(p, accs, pb, pw, ptg, pet)
                    emit_finals(p, accs)

                # ---- loss tail ----
                nc.vector.tensor_reduce(
                    pr1[:], partials[:], mybir.AxisListType.X, ALU.add)
                nc.gpsimd.partition_all_reduce(
                    tot[:], pr1[:], K, bass_isa.ReduceOp.add)
                nc.gpsimd.tensor_scalar_mul(
                    osb[:], tot[0:1, :], float(-WEIGHT / N))
                nc.sync.dma_start(out_d[:], osb[:])

    nc.finalize()
    return nc


_CACHE = {}


def _get_nc():
    if "nc" not in _CACHE:
        _CACHE["nc"] = _build()
    return _CACHE["nc"]


def kernel(images: np.ndarray, segmentations: np.ndarray) -> np.ndarray:
    images = np.ascontiguousarray(np.asarray(images, dtype=np.float32))
    segmentations = np.ascontiguousarray(np.asarray(segmentations, dtype=np.float32))
    assert images.shape == (N, C, H, W) and segmentations.shape == (N, K, 128, 128)
    nc = _get_nc()
    consts = _consts()
    in_maps = [
        {"images": images[n], "segmentations": segmentations[n], **consts}
        for n in range(N)
    ]
    res = run_bass_kernel_spmd(nc, in_maps, list(range(N)))
    total = sum(float(res.results[n]["out"][0]) for n in range(N))
    return np.array([total], dtype=np.float32)


if __name__ == "__main__":
    rng = np.random.RandomState(0)
    img = rng.rand(N, C, H, W).astype(np.float32) * 255.0
    seg = rng.rand(N, K, 128, 128).astype(np.float32)
    print(kernel(img, seg))
